# revision 27
# baseline (speedup 1.0000x reference)
"""Distributed exact inner-product top-k (brute-force kNN) on 8 TRN2 NeuronCores.

Sharding: codebook W is split row-wise into 8 shards of 25000 (one per core);
x is replicated.  Host pre-transposes both so the contraction dim (128) lands
on SBUF partitions.

Device kernel (SPMD, identical graph per core, no collectives):
  - per 1024-wide vocab region (2 PSUM banks): 2x bf16 matmuls [128 rows, 512]
    into PSUM (f32 accumulation); 4 PSUM tiles keep the PE 4 regions ahead
  - each region's 1024 f32 scores are then drained by one of three routes so
    the work is split across the only two engines that can read PSUM:
      A: DVE windowed tensor_reduce(max) [128,256,4] -> 256 window-4 maxima
      C: Act copy PSUM->SBUF bf16, one DVE bf16 fold   -> 512 window-2 maxima
      D: Act copy PSUM->out tile bf16 (raw)            -> 1024 window-1 values
    (GPSIMD/Pool cannot read PSUM on TRN2 and cannot run TensorTensor in this
    toolchain; DMA cannot read PSUM nor max-accumulate — verified empirically)
  - the per-row stream of 13930 window maxima (bf16) is DMA'd out in chunks

Host merge (the all-gather + final top-k of the distributed ANN pattern):
  - per row, select every window whose max clears (128th-largest window max
    - MARGIN); gather those windows' member columns as candidates
  - exact f64 re-rank of the candidates; final top-128 ordered like
    jax.lax.top_k (value desc, index asc)
  - exactness guard: MARGIN >= 2*EPS guarantees containment of the true
    top-128 given |device window max - exact window max| <= EPS; EPS is
    validated per-run on every selected window (device value vs exact f64
    value), and violating rows (expected none) are recomputed exactly.
"""

import numpy as np

B = 1024
D = 128
VOCAB = 200000
NCORES = 8
VSHARD = VOCAB // NCORES  # 25000
REGION = 1024  # 2 PSUM banks of f32
NREG = 24  # full regions per shard
TAIL = VSHARD - NREG * REGION  # 424
TOPK = 128

# Engine route per full region (tail is always 'A'):
#   A = DVE windowed reduce (window 4)
#   C = Act copy + one DVE bf16 fold (window 2)
#   D = Act copy straight to the out tile (window 1, raw)
ROUTES = "DA" * 11 + "DD"
assert len(ROUTES) == NREG

_WINS = {"A": REGION // 4, "C": REGION // 2, "D": REGION}


def _region_wins(r: int) -> int:
    return _WINS[ROUTES[r]] if r < NREG else TAIL // 4


# output offset of each region's windows in the out tile
WOFF = np.concatenate([[0], np.cumsum([_region_wins(r) for r in range(NREG + 1)])])
NWIN = int(WOFF[-1])  # 13930

# |device window max - exact window max| bound: fp8e4 input quantization
# noise (std ~0.6, observed max ~3) + bf16 output quantization (~0.2 at
# score ~45).  Validated at runtime on every selected window.
EPS_BOUND = 3.5
MARGIN = 7.5  # >= 2*EPS_BOUND + slack

LAST_RESULTS = None  # BassKernelResults of the most recent run (for profiling)
_CACHED_NC = None


def build_kernel():
    import concourse.bass as bass  # noqa: F401
    import concourse.tile as tile
    from concourse import bacc, mybir

    F32 = mybir.dt.float32
    BF16 = mybir.dt.bfloat16
    FP8 = mybir.dt.float8e4
    AX = mybir.AxisListType.X
    MAX = mybir.AluOpType.max
    COPY = mybir.ActivationFunctionType.Copy

    nc = bacc.Bacc("TRN2", target_bir_lowering=False, debug=False)
    wt_d = nc.dram_tensor("wt", [D, VSHARD], FP8, kind="ExternalInput")
    xt_d = nc.dram_tensor("xt", [D, B], FP8, kind="ExternalInput")
    out_d = nc.dram_tensor("out_win", [B, NWIN], BF16, kind="ExternalOutput")

    with tile.TileContext(nc) as tc:
        with (
            tc.tile_pool(name="wt", bufs=1) as wt_pool,
            tc.tile_pool(name="xt", bufs=1) as xt_pool,
            tc.tile_pool(name="psum", bufs=4, space="PSUM") as psum_pool,
            tc.tile_pool(name="outw", bufs=4) as out_pool,
        ):
            wt_sb = wt_pool.tile([D, VSHARD], FP8)
            xt_sb = xt_pool.tile([D, B], FP8)
            # xt first: the first matmul's stationary operand should not wait
            # behind the whole 6.4MB W load.  W is split into 16 slabs in
            # consumption order, alternating between the SP HW queue and the
            # gpsimd software-DGE queue (both engines otherwise idle, so the
            # DMA-trigger instruction cost stays off the critical engines).
            nc.sync.dma_start(xt_sb[:], xt_d[:])
            nsplit = 32
            step = VSHARD // nsplit
            for s in range(nsplit):
                hi = VSHARD if s == nsplit - 1 else (s + 1) * step
                eng = nc.sync if s % 2 == 0 else nc.gpsimd
                eng.dma_start(wt_sb[:, s * step:hi], wt_d[:, s * step:hi])

            # out DMA is chunked after these regions so the transfer of a
            # group's early windows overlaps the rest of the group's compute
            DMA_CUTS = {7: (0, int(WOFF[8])),
                        15: (int(WOFF[8]), int(WOFF[16])),
                        NREG: (int(WOFF[16]), NWIN)}

            # Groups are issued pairwise-interleaved: two independent region
            # streams keep both PSUM-drain engines fed through each group's
            # fill and drain phases (kills the per-group pipeline bubbles).
            for gp in range(0, B // 128, 2):
                out_sb0 = out_pool.tile([128, NWIN], BF16, tag="outw")
                out_sb1 = out_pool.tile([128, NWIN], BF16, tag="outw")
                outs = [out_sb0, out_sb1]
                for r in range(NREG + 1):
                    base = r * REGION
                    w_cols = REGION if r < NREG else TAIL
                    route = ROUTES[r] if r < NREG else "A"
                    wo = int(WOFF[r])
                    for gi in range(2):
                        g = gp + gi
                        out_sb = outs[gi]
                        xg = xt_sb[:, g * 128:(g + 1) * 128]
                        ps = psum_pool.tile([128, REGION], F32)
                        for k in range(0, w_cols, 512):
                            kw = min(512, w_cols - k)
                            nc.tensor.matmul(
                                ps[:, k:k + kw],
                                xg,
                                wt_sb[:, base + k:base + k + kw],
                                start=True, stop=True,
                            )
                        owin = out_sb[:, wo:wo + _region_wins(r)]
                        if route == "A":
                            nc.vector.tensor_reduce(
                                owin,
                                ps[:, :w_cols].rearrange(
                                    "p (n w) -> p n w", w=4),
                                axis=AX, op=MAX,
                            )
                        else:  # "D"
                            nc.scalar.activation(owin, ps[:], COPY)
                        if r in DMA_CUTS:
                            lo, hi = DMA_CUTS[r]
                            nc.sync.dma_start(
                                out_d[g * 128:(g + 1) * 128, lo:hi],
                                out_sb[:, lo:hi],
                            )
    nc.compile()
    return nc


def _build_colmap():
    """[NWIN, 4] int64 window->shard-columns map, -1 marks padding."""
    cm = np.full((NWIN, 4), -1, np.int64)
    for r in range(NREG + 1):
        base = r * REGION
        n = _region_wins(r)
        wo = int(WOFF[r])
        route = ROUTES[r] if r < NREG else "A"
        j = np.arange(n)[:, None]
        if route == "A":
            cm[wo:wo + n] = base + 4 * j + np.arange(4)[None, :]
        elif route == "C":
            cm[wo:wo + n, :2] = base + j + np.array([0, REGION // 2])[None, :]
        else:  # D
            cm[wo:wo + n, :1] = base + j
    return cm


_COLMAP = _build_colmap()


def _topk_rows(vals, gidx, k):
    """Per-row top-k ordered like jax.lax.top_k: value desc, index asc."""
    order = np.lexsort((gidx, -vals), axis=-1)[:, :k]
    return (
        np.take_along_axis(gidx, order, axis=1),
        np.take_along_axis(vals, order, axis=1),
    )


def kernel(x: np.ndarray, W: np.ndarray, topk) -> np.ndarray:
    global LAST_RESULTS, _CACHED_NC
    import os

    import ml_dtypes

    from concourse.bass_utils import run_bass_kernel_spmd

    assert x.shape == (B, D) and W.shape == (VOCAB, D)
    assert int(topk) == TOPK
    x = np.ascontiguousarray(np.asarray(x, dtype=np.float32))
    W = np.ascontiguousarray(np.asarray(W, dtype=np.float32))

    if _CACHED_NC is None:
        _CACHED_NC = build_kernel()
    nc = _CACHED_NC

    xt = np.ascontiguousarray(x.T).astype(ml_dtypes.float8_e4m3)
    in_maps = []
    for i in range(NCORES):
        wt_i = np.ascontiguousarray(
            W[i * VSHARD:(i + 1) * VSHARD].T
        ).astype(ml_dtypes.float8_e4m3)
        in_maps.append({"wt": wt_i, "xt": xt})

    LAST_RESULTS = run_bass_kernel_spmd(
        nc,
        in_maps,
        core_ids=list(range(NCORES)),
        trace=bool(int(os.environ.get("KERNEL_TRACE", "0"))),
    )
    results = LAST_RESULTS.results

    # [B, 8*NWIN] device window maxima, f32
    wm = np.concatenate(
        [np.asarray(results[i]["out_win"]).astype(np.float32)
         for i in range(NCORES)], axis=1,
    )
    nwin_all = NCORES * NWIN

    # Per-row window selection: everything >= (128th-largest window max - MARGIN)
    kth = np.partition(wm, nwin_all - TOPK, axis=1)[:, nwin_all - TOPK]
    tau = kth - MARGIN
    counts = (wm >= tau[:, None]).sum(axis=1)
    K = int(min(max(int(counts.max()), TOPK + 64), 6144))
    topw = np.argpartition(-wm, K - 1, axis=1)[:, :K]  # [B, K] window ids

    core_id = topw // NWIN
    wi = topw % NWIN
    cols = _COLMAP[wi]  # [B, K, 4], -1 = pad
    pad = cols < 0
    cand = (np.where(pad, 0, cols) + core_id[..., None] * VSHARD).reshape(B, K * 4)

    # Exact f64 re-rank of the candidate columns (pads scored -inf).
    x64 = x.astype(np.float64)
    W64 = W.astype(np.float64)
    exact = np.empty((B, K * 4), np.float64)
    STEP = 128
    for r0 in range(0, B, STEP):
        r1 = r0 + STEP
        gW = W64[cand[r0:r1]]  # [STEP, K*4, D]
        exact[r0:r1] = np.einsum("bjd,bd->bj", gW, x64[r0:r1])
    exact[pad.reshape(B, K * 4)] = -np.inf

    gidx_top, vals_top = _topk_rows(exact, cand, TOPK)

    # Exactness guards.
    t128 = vals_top[:, -1]
    dev_wmax = np.take_along_axis(wm, topw, axis=1)
    true_wmax = exact.reshape(B, K, 4).max(axis=2)
    err = np.abs(dev_wmax - true_wmax).max(axis=1)
    bad = (
        (err > EPS_BOUND)
        | (tau + EPS_BOUND > t128)
        | (counts > K)
    )
    if os.environ.get("KERNEL_DEBUG"):
        print(f"[kernel] K={K} counts max={counts.max()} "
              f"err max={err.max():.4f} bad rows={int(bad.sum())}")
    for r in np.flatnonzero(bad):
        s = x64[r] @ W64.T
        gidx_top[r] = np.lexsort((np.arange(VOCAB), -s))[:TOPK]

    return gidx_top.astype(np.int32)


# revision 29
# speedup vs baseline: 1.0231x; 1.0231x over previous
"""Distributed exact inner-product top-k (brute-force kNN) on 8 TRN2 NeuronCores.

Sharding: codebook W is split row-wise into 8 shards of 25000 (one per core);
x is replicated.  Host pre-transposes both so the contraction dim (128) lands
on SBUF partitions.

Device kernel (SPMD, identical graph per core, no collectives):
  - per 1024-wide vocab region (2 PSUM banks): 2x bf16 matmuls [128 rows, 512]
    into PSUM (f32 accumulation); 4 PSUM tiles keep the PE 4 regions ahead
  - each region's 1024 f32 scores are then drained by one of three routes so
    the work is split across the only two engines that can read PSUM:
      A: DVE windowed tensor_reduce(max) [128,256,4] -> 256 window-4 maxima
      C: Act copy PSUM->SBUF bf16, one DVE bf16 fold   -> 512 window-2 maxima
      D: Act copy PSUM->out tile bf16 (raw)            -> 1024 window-1 values
    (GPSIMD/Pool cannot read PSUM on TRN2 and cannot run TensorTensor in this
    toolchain; DMA cannot read PSUM nor max-accumulate — verified empirically)
  - the per-row stream of 13930 window maxima (bf16) is DMA'd out in chunks

Host merge (the all-gather + final top-k of the distributed ANN pattern):
  - per row, select every window whose max clears (128th-largest window max
    - MARGIN); gather those windows' member columns as candidates
  - exact f64 re-rank of the candidates; final top-128 ordered like
    jax.lax.top_k (value desc, index asc)
  - exactness guard: MARGIN >= 2*EPS guarantees containment of the true
    top-128 given |device window max - exact window max| <= EPS; EPS is
    validated per-run on every selected window (device value vs exact f64
    value), and violating rows (expected none) are recomputed exactly.
"""

import numpy as np

B = 1024
D = 128
VOCAB = 200000
NCORES = 8
VSHARD = VOCAB // NCORES  # 25000
REGION = 1024  # 2 PSUM banks of f32
NREG = 24  # full regions per shard
TAIL = VSHARD - NREG * REGION  # 424
TOPK = 128

# Engine route per full region (tail is always 'A'):
#   A = DVE windowed reduce (window 4)
#   C = Act copy + one DVE bf16 fold (window 2)
#   D = Act copy straight to the out tile (window 1, raw)
ROUTES = "DA" * 11 + "DD"
assert len(ROUTES) == NREG

_WINS = {"A": REGION // 4, "C": REGION // 2, "D": REGION}


def _region_wins(r: int) -> int:
    return _WINS[ROUTES[r]] if r < NREG else TAIL // 4


# output offset of each region's windows in the out tile
WOFF = np.concatenate([[0], np.cumsum([_region_wins(r) for r in range(NREG + 1)])])
NWIN = int(WOFF[-1])  # 13930

# |device window max - exact window max| bound: fp8e4 input quantization
# noise (std ~0.6, observed max ~3) + bf16 output quantization (~0.2 at
# score ~45).  Validated at runtime on every selected window.
EPS_BOUND = 3.5
MARGIN = 7.5  # >= 2*EPS_BOUND + slack

LAST_RESULTS = None  # BassKernelResults of the most recent run (for profiling)
_CACHED_NC = None


def build_kernel():
    import concourse.bass as bass  # noqa: F401
    import concourse.tile as tile
    from concourse import bacc, mybir

    F32 = mybir.dt.float32
    BF16 = mybir.dt.bfloat16
    FP8 = mybir.dt.float8e4
    AX = mybir.AxisListType.X
    MAX = mybir.AluOpType.max
    COPY = mybir.ActivationFunctionType.Copy

    nc = bacc.Bacc("TRN2", target_bir_lowering=False, debug=False)
    wt_d = nc.dram_tensor("wt", [D, VSHARD], FP8, kind="ExternalInput")
    xt_d = nc.dram_tensor("xt", [D, B], FP8, kind="ExternalInput")
    out_d = nc.dram_tensor("out_win", [B, NWIN], BF16, kind="ExternalOutput")

    with tile.TileContext(nc) as tc:
        with (
            tc.tile_pool(name="wt", bufs=1) as wt_pool,
            tc.tile_pool(name="xt", bufs=1) as xt_pool,
            tc.tile_pool(name="psum", bufs=4, space="PSUM") as psum_pool,
            tc.tile_pool(name="outw", bufs=4) as out_pool,
        ):
            wt_sb = wt_pool.tile([D, VSHARD], FP8)
            xt_sb = xt_pool.tile([D, B], FP8)
            # xt first: the first matmul's stationary operand should not wait
            # behind the whole 6.4MB W load.  W is split into 16 slabs in
            # consumption order, alternating between the SP HW queue and the
            # gpsimd software-DGE queue (both engines otherwise idle, so the
            # DMA-trigger instruction cost stays off the critical engines).
            nc.sync.dma_start(xt_sb[:], xt_d[:])
            nsplit = 32
            step = VSHARD // nsplit
            for s in range(nsplit):
                hi = VSHARD if s == nsplit - 1 else (s + 1) * step
                eng = nc.sync if s % 2 == 0 else nc.gpsimd
                eng.dma_start(wt_sb[:, s * step:hi], wt_d[:, s * step:hi])

            # out DMA is chunked after these regions so the transfer of a
            # group's early windows overlaps the rest of the group's compute;
            # cuts alternate between the SP and gpsimd queues by group parity
            # so the two output streams drain in parallel
            cut_regs = [2, 5, 8, 11, 14, 17, 20, NREG]
            DMA_CUTS = {}
            prev = 0
            for cr in cut_regs:
                DMA_CUTS[cr] = (prev, int(WOFF[cr + 1]))
                prev = int(WOFF[cr + 1])

            # Groups are issued pairwise-interleaved: two independent region
            # streams keep both PSUM-drain engines fed through each group's
            # fill and drain phases (kills the per-group pipeline bubbles).
            for gp in range(0, B // 128, 2):
                out_sb0 = out_pool.tile([128, NWIN], BF16, tag="outw")
                out_sb1 = out_pool.tile([128, NWIN], BF16, tag="outw")
                outs = [out_sb0, out_sb1]
                for r in range(NREG + 1):
                    base = r * REGION
                    w_cols = REGION if r < NREG else TAIL
                    route = ROUTES[r] if r < NREG else "A"
                    wo = int(WOFF[r])
                    for gi in range(2):
                        g = gp + gi
                        out_sb = outs[gi]
                        xg = xt_sb[:, g * 128:(g + 1) * 128]
                        ps = psum_pool.tile([128, REGION], F32)
                        for k in range(0, w_cols, 512):
                            kw = min(512, w_cols - k)
                            nc.tensor.matmul(
                                ps[:, k:k + kw],
                                xg,
                                wt_sb[:, base + k:base + k + kw],
                                start=True, stop=True,
                            )
                        owin = out_sb[:, wo:wo + _region_wins(r)]
                        if route == "A":
                            nc.vector.tensor_reduce(
                                owin,
                                ps[:, :w_cols].rearrange(
                                    "p (n w) -> p n w", w=4),
                                axis=AX, op=MAX,
                            )
                        else:  # "D"
                            nc.scalar.activation(owin, ps[:], COPY)
                        if r in DMA_CUTS:
                            lo, hi = DMA_CUTS[r]
                            eng = nc.sync if gi == 0 else nc.gpsimd
                            eng.dma_start(
                                out_d[g * 128:(g + 1) * 128, lo:hi],
                                out_sb[:, lo:hi],
                            )
    nc.compile()
    return nc


def _build_colmap():
    """[NWIN, 4] int64 window->shard-columns map, -1 marks padding."""
    cm = np.full((NWIN, 4), -1, np.int64)
    for r in range(NREG + 1):
        base = r * REGION
        n = _region_wins(r)
        wo = int(WOFF[r])
        route = ROUTES[r] if r < NREG else "A"
        j = np.arange(n)[:, None]
        if route == "A":
            cm[wo:wo + n] = base + 4 * j + np.arange(4)[None, :]
        elif route == "C":
            cm[wo:wo + n, :2] = base + j + np.array([0, REGION // 2])[None, :]
        else:  # D
            cm[wo:wo + n, :1] = base + j
    return cm


_COLMAP = _build_colmap()


def _topk_rows(vals, gidx, k):
    """Per-row top-k ordered like jax.lax.top_k: value desc, index asc."""
    order = np.lexsort((gidx, -vals), axis=-1)[:, :k]
    return (
        np.take_along_axis(gidx, order, axis=1),
        np.take_along_axis(vals, order, axis=1),
    )


def kernel(x: np.ndarray, W: np.ndarray, topk) -> np.ndarray:
    global LAST_RESULTS, _CACHED_NC
    import os

    import ml_dtypes

    from concourse.bass_utils import run_bass_kernel_spmd

    assert x.shape == (B, D) and W.shape == (VOCAB, D)
    assert int(topk) == TOPK
    x = np.ascontiguousarray(np.asarray(x, dtype=np.float32))
    W = np.ascontiguousarray(np.asarray(W, dtype=np.float32))

    if _CACHED_NC is None:
        _CACHED_NC = build_kernel()
    nc = _CACHED_NC

    xt = np.ascontiguousarray(x.T).astype(ml_dtypes.float8_e4m3)
    in_maps = []
    for i in range(NCORES):
        wt_i = np.ascontiguousarray(
            W[i * VSHARD:(i + 1) * VSHARD].T
        ).astype(ml_dtypes.float8_e4m3)
        in_maps.append({"wt": wt_i, "xt": xt})

    LAST_RESULTS = run_bass_kernel_spmd(
        nc,
        in_maps,
        core_ids=list(range(NCORES)),
        trace=bool(int(os.environ.get("KERNEL_TRACE", "0"))),
    )
    results = LAST_RESULTS.results

    # [B, 8*NWIN] device window maxima, f32
    wm = np.concatenate(
        [np.asarray(results[i]["out_win"]).astype(np.float32)
         for i in range(NCORES)], axis=1,
    )
    nwin_all = NCORES * NWIN

    # Per-row window selection: everything >= (128th-largest window max - MARGIN)
    kth = np.partition(wm, nwin_all - TOPK, axis=1)[:, nwin_all - TOPK]
    tau = kth - MARGIN
    counts = (wm >= tau[:, None]).sum(axis=1)
    K = int(min(max(int(counts.max()), TOPK + 64), 6144))
    topw = np.argpartition(-wm, K - 1, axis=1)[:, :K]  # [B, K] window ids

    core_id = topw // NWIN
    wi = topw % NWIN
    cols = _COLMAP[wi]  # [B, K, 4], -1 = pad
    pad = cols < 0
    cand = (np.where(pad, 0, cols) + core_id[..., None] * VSHARD).reshape(B, K * 4)

    # Exact f64 re-rank of the candidate columns (pads scored -inf).
    x64 = x.astype(np.float64)
    W64 = W.astype(np.float64)
    exact = np.empty((B, K * 4), np.float64)
    STEP = 128
    for r0 in range(0, B, STEP):
        r1 = r0 + STEP
        gW = W64[cand[r0:r1]]  # [STEP, K*4, D]
        exact[r0:r1] = np.einsum("bjd,bd->bj", gW, x64[r0:r1])
    exact[pad.reshape(B, K * 4)] = -np.inf

    gidx_top, vals_top = _topk_rows(exact, cand, TOPK)

    # Exactness guards.
    t128 = vals_top[:, -1]
    dev_wmax = np.take_along_axis(wm, topw, axis=1)
    true_wmax = exact.reshape(B, K, 4).max(axis=2)
    err = np.abs(dev_wmax - true_wmax).max(axis=1)
    bad = (
        (err > EPS_BOUND)
        | (tau + EPS_BOUND > t128)
        | (counts > K)
    )
    if os.environ.get("KERNEL_DEBUG"):
        print(f"[kernel] K={K} counts max={counts.max()} "
              f"err max={err.max():.4f} bad rows={int(bad.sum())}")
    for r in np.flatnonzero(bad):
        s = x64[r] @ W64.T
        gidx_top[r] = np.lexsort((np.arange(VOCAB), -s))[:TOPK]

    return gidx_top.astype(np.int32)


# revision 34
# speedup vs baseline: 1.0242x; 1.0011x over previous
"""Distributed exact inner-product top-k (brute-force kNN) on 8 TRN2 NeuronCores.

Sharding: codebook W is split row-wise into 8 shards of 25000 (one per core);
x is replicated.  Host pre-transposes both so the contraction dim (128) lands
on SBUF partitions.

Device kernel (SPMD, identical graph per core, no collectives):
  - per 1024-wide vocab region (2 PSUM banks): 2x bf16 matmuls [128 rows, 512]
    into PSUM (f32 accumulation); 4 PSUM tiles keep the PE 4 regions ahead
  - each region's 1024 f32 scores are then drained by one of three routes so
    the work is split across the only two engines that can read PSUM:
      A: DVE windowed tensor_reduce(max) [128,256,4] -> 256 window-4 maxima
      C: Act copy PSUM->SBUF bf16, one DVE bf16 fold   -> 512 window-2 maxima
      D: Act copy PSUM->out tile bf16 (raw)            -> 1024 window-1 values
    (GPSIMD/Pool cannot read PSUM on TRN2 and cannot run TensorTensor in this
    toolchain; DMA cannot read PSUM nor max-accumulate — verified empirically)
  - the per-row stream of 13930 window maxima (bf16) is DMA'd out in chunks

Host merge (the all-gather + final top-k of the distributed ANN pattern):
  - per row, select every window whose max clears (128th-largest window max
    - MARGIN); gather those windows' member columns as candidates
  - exact f64 re-rank of the candidates; final top-128 ordered like
    jax.lax.top_k (value desc, index asc)
  - exactness guard: MARGIN >= 2*EPS guarantees containment of the true
    top-128 given |device window max - exact window max| <= EPS; EPS is
    validated per-run on every selected window (device value vs exact f64
    value), and violating rows (expected none) are recomputed exactly.
"""

import numpy as np

B = 1024
D = 128
VOCAB = 200000
NCORES = 8
VSHARD = VOCAB // NCORES  # 25000
REGION = 1024  # 2 PSUM banks of f32
NREG = 24  # full regions per shard
TAIL = VSHARD - NREG * REGION  # 424
TOPK = 128

# Engine route per full region (tail is always 'A'):
#   A = DVE windowed reduce (window 4)
#   C = Act copy + one DVE bf16 fold (window 2)
#   D = Act copy straight to the out tile (window 1, raw)
ROUTES = "DA" * 11 + "DD"
assert len(ROUTES) == NREG

_WINS = {"A": REGION // 4, "C": REGION // 2, "D": REGION}


def _region_wins(r: int) -> int:
    return _WINS[ROUTES[r]] if r < NREG else TAIL // 4


# output offset of each region's windows in the out tile
WOFF = np.concatenate([[0], np.cumsum([_region_wins(r) for r in range(NREG + 1)])])
NWIN = int(WOFF[-1])  # 13930

# |device window max - exact window max| bound: fp8e4 input quantization
# noise (std ~0.6, observed max ~3) + fp8e4 output quantization (+-2 at
# score ~40-64).  Validated at runtime on every selected window.
EPS_BOUND = 6.0
MARGIN = 12.5  # >= 2*EPS_BOUND + slack

LAST_RESULTS = None  # BassKernelResults of the most recent run (for profiling)
_CACHED_NC = None


def build_kernel():
    import concourse.bass as bass  # noqa: F401
    import concourse.tile as tile
    from concourse import bacc, mybir

    F32 = mybir.dt.float32
    BF16 = mybir.dt.bfloat16
    FP8 = mybir.dt.float8e4
    AX = mybir.AxisListType.X
    MAX = mybir.AluOpType.max
    COPY = mybir.ActivationFunctionType.Copy

    nc = bacc.Bacc("TRN2", target_bir_lowering=False, debug=False)
    wt_d = nc.dram_tensor("wt", [D, VSHARD], FP8, kind="ExternalInput")
    xt_d = nc.dram_tensor("xt", [D, B], FP8, kind="ExternalInput")
    out_d = nc.dram_tensor("out_win", [B, NWIN], FP8, kind="ExternalOutput")

    with tile.TileContext(nc) as tc:
        with (
            tc.tile_pool(name="wt", bufs=1) as wt_pool,
            tc.tile_pool(name="xt", bufs=1) as xt_pool,
            tc.tile_pool(name="psum", bufs=4, space="PSUM") as psum_pool,
            tc.tile_pool(name="outw", bufs=4) as out_pool,
        ):
            wt_sb = wt_pool.tile([D, VSHARD], FP8)
            xt_sb = xt_pool.tile([D, B], FP8)
            # xt first: the first matmul's stationary operand should not wait
            # behind the whole 6.4MB W load.  W is split into 16 slabs in
            # consumption order, alternating between the SP HW queue and the
            # gpsimd software-DGE queue (both engines otherwise idle, so the
            # DMA-trigger instruction cost stays off the critical engines).
            nc.sync.dma_start(xt_sb[:], xt_d[:])
            nsplit = 32
            step = VSHARD // nsplit
            for s in range(nsplit):
                hi = VSHARD if s == nsplit - 1 else (s + 1) * step
                eng = nc.sync if s % 2 == 0 else nc.gpsimd
                eng.dma_start(wt_sb[:, s * step:hi], wt_d[:, s * step:hi])

            # out DMA is chunked after these regions so the transfer of a
            # group's early windows overlaps the rest of the group's compute;
            # cuts alternate between the SP and gpsimd queues by group parity
            # so the two output streams drain in parallel
            cut_regs = [2, 5, 8, 11, 14, 17, 20, NREG]
            DMA_CUTS = {}
            prev = 0
            for cr in cut_regs:
                DMA_CUTS[cr] = (prev, int(WOFF[cr + 1]))
                prev = int(WOFF[cr + 1])

            # Groups are issued pairwise-interleaved: two independent region
            # streams keep both PSUM-drain engines fed through each group's
            # fill and drain phases (kills the per-group pipeline bubbles).
            for gp in range(0, B // 128, 2):
                out_sb0 = out_pool.tile([128, NWIN], FP8, tag="outw")
                out_sb1 = out_pool.tile([128, NWIN], FP8, tag="outw")
                outs = [out_sb0, out_sb1]
                for r in range(NREG + 1):
                    base = r * REGION
                    w_cols = REGION if r < NREG else TAIL
                    route = ROUTES[r] if r < NREG else "A"
                    wo = int(WOFF[r])
                    for gi in range(2):
                        g = gp + gi
                        out_sb = outs[gi]
                        xg = xt_sb[:, g * 128:(g + 1) * 128]
                        ps = psum_pool.tile([128, REGION], F32)
                        for k in range(0, w_cols, 512):
                            kw = min(512, w_cols - k)
                            nc.tensor.matmul(
                                ps[:, k:k + kw],
                                xg,
                                wt_sb[:, base + k:base + k + kw],
                                start=True, stop=True,
                            )
                        owin = out_sb[:, wo:wo + _region_wins(r)]
                        if route == "A":
                            nc.vector.tensor_reduce(
                                owin,
                                ps[:, :w_cols].rearrange(
                                    "p (n w) -> p n w", w=4),
                                axis=AX, op=MAX,
                            )
                        else:  # "D"
                            nc.scalar.activation(owin, ps[:], COPY)
                        if r in DMA_CUTS:
                            lo, hi = DMA_CUTS[r]
                            eng = nc.sync if gi == 0 else nc.gpsimd
                            eng.dma_start(
                                out_d[g * 128:(g + 1) * 128, lo:hi],
                                out_sb[:, lo:hi],
                            )
    nc.compile()
    return nc


def _build_colmap():
    """[NWIN, 4] int64 window->shard-columns map, -1 marks padding."""
    cm = np.full((NWIN, 4), -1, np.int64)
    for r in range(NREG + 1):
        base = r * REGION
        n = _region_wins(r)
        wo = int(WOFF[r])
        route = ROUTES[r] if r < NREG else "A"
        j = np.arange(n)[:, None]
        if route == "A":
            cm[wo:wo + n] = base + 4 * j + np.arange(4)[None, :]
        elif route == "C":
            cm[wo:wo + n, :2] = base + j + np.array([0, REGION // 2])[None, :]
        else:  # D
            cm[wo:wo + n, :1] = base + j
    return cm


_COLMAP = _build_colmap()


def _topk_rows(vals, gidx, k):
    """Per-row top-k ordered like jax.lax.top_k: value desc, index asc."""
    order = np.lexsort((gidx, -vals), axis=-1)[:, :k]
    return (
        np.take_along_axis(gidx, order, axis=1),
        np.take_along_axis(vals, order, axis=1),
    )


def kernel(x: np.ndarray, W: np.ndarray, topk) -> np.ndarray:
    global LAST_RESULTS, _CACHED_NC
    import os

    import ml_dtypes

    from concourse.bass_utils import run_bass_kernel_spmd

    assert x.shape == (B, D) and W.shape == (VOCAB, D)
    assert int(topk) == TOPK
    x = np.ascontiguousarray(np.asarray(x, dtype=np.float32))
    W = np.ascontiguousarray(np.asarray(W, dtype=np.float32))

    if _CACHED_NC is None:
        _CACHED_NC = build_kernel()
    nc = _CACHED_NC

    xt = np.ascontiguousarray(x.T).astype(ml_dtypes.float8_e4m3)
    in_maps = []
    for i in range(NCORES):
        wt_i = np.ascontiguousarray(
            W[i * VSHARD:(i + 1) * VSHARD].T
        ).astype(ml_dtypes.float8_e4m3)
        in_maps.append({"wt": wt_i, "xt": xt})

    LAST_RESULTS = run_bass_kernel_spmd(
        nc,
        in_maps,
        core_ids=list(range(NCORES)),
        trace=bool(int(os.environ.get("KERNEL_TRACE", "0"))),
    )
    results = LAST_RESULTS.results

    # [B, 8*NWIN] device window maxima, f32
    wm = np.concatenate(
        [np.asarray(results[i]["out_win"]).astype(np.float32)
         for i in range(NCORES)], axis=1,
    )
    nwin_all = NCORES * NWIN

    # Per-row window selection: everything >= (128th-largest window max - MARGIN)
    kth = np.partition(wm, nwin_all - TOPK, axis=1)[:, nwin_all - TOPK]
    tau = kth - MARGIN
    counts = (wm >= tau[:, None]).sum(axis=1)
    K = int(min(max(int(counts.max()), TOPK + 64), 8192))
    topw = np.argpartition(-wm, K - 1, axis=1)[:, :K]  # [B, K] window ids

    core_id = topw // NWIN
    wi = topw % NWIN
    cols = _COLMAP[wi]  # [B, K, 4], -1 = pad
    pad = cols < 0
    cand = (np.where(pad, 0, cols) + core_id[..., None] * VSHARD).reshape(B, K * 4)

    # Exact f64 re-rank of the candidate columns (pads scored -inf).
    x64 = x.astype(np.float64)
    W64 = W.astype(np.float64)
    exact = np.empty((B, K * 4), np.float64)
    STEP = 32
    for r0 in range(0, B, STEP):
        r1 = r0 + STEP
        gW = W64[cand[r0:r1]]  # [STEP, K*4, D]
        exact[r0:r1] = np.einsum("bjd,bd->bj", gW, x64[r0:r1])
    exact[pad.reshape(B, K * 4)] = -np.inf

    gidx_top, vals_top = _topk_rows(exact, cand, TOPK)

    # Exactness guards.
    t128 = vals_top[:, -1]
    dev_wmax = np.take_along_axis(wm, topw, axis=1)
    true_wmax = exact.reshape(B, K, 4).max(axis=2)
    err = np.abs(dev_wmax - true_wmax).max(axis=1)
    bad = (
        (err > EPS_BOUND)
        | (tau + EPS_BOUND > t128)
        | (counts > K)
    )
    if os.environ.get("KERNEL_DEBUG"):
        print(f"[kernel] K={K} counts max={counts.max()} "
              f"err max={err.max():.4f} bad rows={int(bad.sum())}")
    for r in np.flatnonzero(bad):
        s = x64[r] @ W64.T
        gidx_top[r] = np.lexsort((np.arange(VOCAB), -s))[:TOPK]

    return gidx_top.astype(np.int32)
